# revision 1
# baseline (speedup 1.0000x reference)
"""Trainium2 Bass kernel for nn_Block_77481210020339 (HEALPix Swin-style block).

Pipeline: ff1(256->512)+gelu -> [LN -> win16-attn -> +res -> LN -> MLP(512->2048->512) -> +res] (block A)
          -> same shifted by ws/2=8 (block B) -> spectral-norm linear (512->256).

Sharding: 8 cores, each takes half of one batch image (24576 of 49152 pixels)
plus a redundantly-computed halo, so no collectives are needed.  Each core runs
two phases over 512-token slabs:
  phase 1: x -> ff1 -> block A -> h1 (HBM scratch, bf16 [c,t] layout)
  phase 2: h1 -> block B (shifted windows = aligned windows on slabs offset -8)
           -> ff2 (spectral-norm folded on host) -> out

Throughput rework vs the f32 baseline (device exec 12.6ms -> ~8ms/core):
  - x is pre-transposed to [c,t] and cast to bf16 on the host: no PE input
    transposes, all matmuls bf16 (FWL weight loads).
  - the whole residual stream and HBM scratch are bf16 (f32 only inside PSUM
    and LN statistics), doubling DVE throughput and halving DMA bytes.
  - attention: one batched Exp over all 8 heads' scores [128,1024], bf16
    probabilities, single batched P^T eviction, fast reciprocal.
  - LN split into stats (PE matmuls) and a serial scalar/vector tail that is
    emitted only after independent PE work (the PE queue is in-order, so a
    matmul waiting on the tail would head-of-line-block everything after it).
  - 4-deep software pipeline per phase: front/LN1(i), qkv+attention(i-1)
    with m2(i-3) matmuls interleaved into the softmax chain (m2 evicts on
    the vector engine, so the scalar queue - busy with Exp + act-table
    loads - never gates PSUM recycling), then proj(i-1), m1(i-2) as one
    contiguous Gelu block, LN2(i-1).  Keeps the PE fed through every
    LN/softmax chain so the HAM clock gate stays at 8/8.
  - exact 8-bank PSUM plan: mmout x2, stats x2, S x2, PT(bf16) x1, OT x1.
"""

import sys

sys.path.insert(0, "/opt/trn_rl_repo")

from contextlib import ExitStack

import numpy as np

import concourse.bass as bass
from concourse import bacc
import concourse.mybir as mybir
import concourse.tile as tile
from concourse.bass_utils import run_bass_kernel_spmd

F32 = mybir.dt.float32
BF16 = mybir.dt.bfloat16
AF = mybir.ActivationFunctionType
ALU = mybir.AluOpType
GELU_FN = AF.Gelu

# problem dims
B, N, CIN, CHID, COUT, WS, NH, HD = 4, 49152, 256, 512, 256, 16, 8, 64
CH4 = 4 * CHID  # 2048
P = 128
SLAB = 512
KC = CHID // P  # 4 channel chunks of the stream
JC = CIN // P   # 2 input channel chunks
TC = SLAB // P  # 4 token chunks per slab
HALO = 256


class Cfg:
    def __init__(self, t_out=24576, n_cores=8):
        self.t_out = t_out
        self.t_in = t_out + 2 * SLAB  # phase-1 scratch tokens
        self.n1 = self.t_in // SLAB  # phase-1 slabs
        self.n2 = t_out // SLAB + 1  # phase-2 slabs
        self.n_cores = n_cores


# ---------------------------------------------------------------------------
# program builder
# ---------------------------------------------------------------------------


def build_program(cfg: Cfg):
    nc = bacc.Bacc("TRN2", target_bir_lowering=False, debug=False,
                   enable_asserts=True, num_devices=cfg.n_cores)

    # ---- external params -------------------------------------------------
    x_in = nc.declare_dram_parameter("x_in", [JC, P, cfg.t_in], BF16, isOutput=False)
    wff1 = nc.declare_dram_parameter("wff1", [JC, P, CHID], BF16, isOutput=False)
    bff1 = nc.declare_dram_parameter("bff1", [P, KC], F32, isOutput=False)
    wff2 = nc.declare_dram_parameter("wff2", [KC, P, COUT], BF16, isOutput=False)
    bff2row = nc.declare_dram_parameter("bff2row", [1, COUT], BF16, isOutput=False)

    blk = {}
    for p in ("a", "b"):
        blk[p] = dict(
            wqkv=nc.declare_dram_parameter(f"{p}_wqkv", [KC, P, 3 * CHID], BF16, isOutput=False),
            bqk=nc.declare_dram_parameter(f"{p}_bqk", [P, 2 * KC], F32, isOutput=False),
            bvrow=nc.declare_dram_parameter(f"{p}_bvrow", [1, CHID], BF16, isOutput=False),
            wpw=nc.declare_dram_parameter(f"{p}_wpw", [KC, P, CHID], BF16, isOutput=False),
            bpb=nc.declare_dram_parameter(f"{p}_bpb", [P, KC], F32, isOutput=False),
            wm1=nc.declare_dram_parameter(f"{p}_wm1", [KC, P, CH4], BF16, isOutput=False),
            bm1=nc.declare_dram_parameter(f"{p}_bm1", [P, CH4 // P], F32, isOutput=False),
            wm2=nc.declare_dram_parameter(f"{p}_wm2", [CH4 // P, P, CHID], BF16, isOutput=False),
            bm2=nc.declare_dram_parameter(f"{p}_bm2", [P, KC], F32, isOutput=False),
        )

    ident_bf = nc.declare_dram_parameter("ident_bf", [P, P], BF16, isOutput=False)
    mask_bf = nc.declare_dram_parameter("mask_bf", [P, P], BF16, isOutput=False)
    ones_col_bf = nc.declare_dram_parameter("ones_col_bf", [P, 1], BF16, isOutput=False)
    ones_row_bf = nc.declare_dram_parameter("ones_row_bf", [1, P], BF16, isOutput=False)
    eps_t = nc.declare_dram_parameter("eps_t", [1, 1], F32, isOutput=False)

    out = nc.declare_dram_parameter("out", [cfg.t_out, COUT], F32, isOutput=True)

    ctx = ExitStack()
    with tile.TileContext(nc) as tc:
        with ctx:
            cpool = ctx.enter_context(tc.tile_pool(name="const", bufs=1))
            wpool = ctx.enter_context(tc.tile_pool(name="w", bufs=1))
            a1 = ctx.enter_context(tc.tile_pool(name="a1", bufs=2))
            a2 = ctx.enter_context(tc.tile_pool(name="a2", bufs=2))
            dpool = ctx.enter_context(tc.tile_pool(name="dram", bufs=1, space="DRAM"))
            ps2 = ctx.enter_context(tc.tile_pool(name="ps2", bufs=2, space="PSUM"))
            ps1 = ctx.enter_context(tc.tile_pool(name="ps1", bufs=1, space="PSUM"))

            # ---- constants ----
            ident_t = cpool.tile([P, P], BF16, name="ident_t")
            nc.sync.dma_start(out=ident_t[:], in_=ident_bf[:])
            mask_t = cpool.tile([P, P], BF16, name="mask_t")
            nc.sync.dma_start(out=mask_t[:], in_=mask_bf[:])
            onescol_t = cpool.tile([P, 1], BF16, name="onescol_t")
            nc.sync.dma_start(out=onescol_t[:], in_=ones_col_bf[:])
            onesrow_t = cpool.tile([1, P], BF16, name="onesrow_t")
            nc.sync.dma_start(out=onesrow_t[:], in_=ones_row_bf[:])
            epsc_t = cpool.tile([1, 1], F32, name="epsc_t")
            nc.sync.dma_start(out=epsc_t[:], in_=eps_t[:])
            wff1_t = cpool.tile([P, JC * CHID], BF16, name="wff1_t")
            for j in range(JC):
                nc.sync.dma_start(out=wff1_t[:, j * CHID:(j + 1) * CHID], in_=wff1[j])
            bff1_t = cpool.tile([P, KC], F32, name="bff1_t")
            nc.sync.dma_start(out=bff1_t[:], in_=bff1[:])
            wff2_t = cpool.tile([P, KC * COUT], BF16, name="wff2_t")
            for j in range(KC):
                nc.sync.dma_start(out=wff2_t[:, j * COUT:(j + 1) * COUT], in_=wff2[j])
            bff2_t = cpool.tile([1, COUT], BF16, name="bff2_t")
            nc.sync.dma_start(out=bff2_t[:], in_=bff2row[:])

            # ---- HBM scratch for block-A output (bf16) ----
            h1_t = dpool.tile([KC, P, cfg.t_in], BF16, name="h1_t")

            # ---------------------------------------------------------------
            def load_block_weights(p):
                w = blk[p]
                t = {}
                t["wqkv"] = [wpool.tile([P, 3 * CHID], BF16, name=f"wqkv{k}", tag=f"wqkv{k}") for k in range(KC)]
                for k in range(KC):
                    nc.sync.dma_start(out=t["wqkv"][k][:], in_=w["wqkv"][k])
                t["bqk"] = wpool.tile([P, 2 * KC], F32, name="bqk", tag="bqk")
                nc.sync.dma_start(out=t["bqk"][:], in_=w["bqk"][:])
                t["bvrow"] = wpool.tile([1, CHID], BF16, name="bvrow", tag="bvrow")
                nc.sync.dma_start(out=t["bvrow"][:], in_=w["bvrow"][:])
                t["wpw"] = [wpool.tile([P, CHID], BF16, name=f"wpw{k}", tag=f"wpw{k}") for k in range(KC)]
                for k in range(KC):
                    nc.sync.dma_start(out=t["wpw"][k][:], in_=w["wpw"][k])
                t["bpb"] = wpool.tile([P, KC], F32, name="bpb", tag="bpb")
                nc.sync.dma_start(out=t["bpb"][:], in_=w["bpb"][:])
                t["wm1"] = [wpool.tile([P, CH4], BF16, name=f"wm1{k}", tag=f"wm1{k}") for k in range(KC)]
                for k in range(KC):
                    nc.sync.dma_start(out=t["wm1"][k][:], in_=w["wm1"][k])
                t["bm1"] = wpool.tile([P, CH4 // P], F32, name="bm1", tag="bm1")
                nc.sync.dma_start(out=t["bm1"][:], in_=w["bm1"][:])
                t["wm2"] = [wpool.tile([P, CHID], BF16, name=f"wm2{k}", tag=f"wm2{k}") for k in range(CH4 // P)]
                for k in range(CH4 // P):
                    nc.sync.dma_start(out=t["wm2"][k][:], in_=w["wm2"][k])
                t["bm2"] = wpool.tile([P, KC], F32, name="bm2", tag="bm2")
                nc.sync.dma_start(out=t["bm2"][:], in_=w["bm2"][:])
                return t

            # ---------------------------------------------------------------
            # LayerNorm split in two so the serial scalar/vector tail can be
            # emitted AFTER independent PE work (the PE queue is in-order; a
            # matmul that waits on the tail would head-of-line-block every
            # matmul emitted after it).
            def ln_stats(h_t, nm):
                sq = a1.tile([P, KC * SLAB], BF16, name=f"{nm}_sq", tag="ln_sq", bufs=1)
                for k in range(KC):
                    sl = slice(k * SLAB, (k + 1) * SLAB)
                    nc.vector.tensor_mul(sq[:, sl], h_t[:, sl], h_t[:, sl])
                st = ps1.tile([64, SLAB], F32, name=f"{nm}_st", tag="stats", bufs=2)
                for k in range(KC):
                    sl = slice(k * SLAB, (k + 1) * SLAB)
                    nc.tensor.matmul(st[0:1, :], onescol_t[:], h_t[:, sl],
                                     start=(k == 0), stop=(k == KC - 1))
                for k in range(KC):
                    sl = slice(k * SLAB, (k + 1) * SLAB)
                    nc.tensor.matmul(st[32:33, :], onescol_t[:], sq[:, sl],
                                     start=(k == 0), stop=(k == KC - 1))
                return st

            def ln_apply(h_t, st, nm, ypool, ytag):
                m_row = a1.tile([1, SLAB], F32, name=f"{nm}_m", tag="ln_m", bufs=1)
                nc.vector.tensor_scalar(m_row[:], st[0:1, :], 1.0 / CHID, None,
                                        op0=ALU.mult)
                ms = a1.tile([1, SLAB], F32, name=f"{nm}_ms", tag="ln_ms", bufs=1)
                nc.vector.tensor_mul(ms[:], m_row[:], m_row[:])
                var = a1.tile([1, SLAB], F32, name=f"{nm}_var", tag="ln_var", bufs=1)
                nc.vector.scalar_tensor_tensor(var[:], st[32:33, :], 1.0 / CHID, ms[:],
                                               op0=ALU.mult, op1=ALU.subtract)
                std = a1.tile([1, SLAB], F32, name=f"{nm}_std", tag="ln_std", bufs=1)
                nc.scalar.activation(std[:], var[:], AF.Sqrt, bias=epsc_t[0:1, 0:1])
                r = a1.tile([1, SLAB], F32, name=f"{nm}_r", tag="ln_r", bufs=1)
                nc.vector.reciprocal_approx_fast(out=r[:], in_=std[:])
                rbf = a1.tile([1, SLAB], BF16, name=f"{nm}_rbf", tag="ln_rbf", bufs=1)
                nc.vector.tensor_scalar(rbf[:], r[:], 1.0, None, op0=ALU.mult)
                mrbf = a1.tile([1, SLAB], BF16, name=f"{nm}_mrbf", tag="ln_mrbf", bufs=1)
                nc.vector.scalar_tensor_tensor(mrbf[:], st[0:1, :], 1.0 / CHID, r[:],
                                               op0=ALU.mult, op1=ALU.mult)
                rb_ps = ps2.tile([P, SLAB], F32, name=f"{nm}_rb", tag="mmout")
                nc.tensor.matmul(rb_ps[:], onesrow_t[:], rbf[:])
                mrb_ps = ps2.tile([P, SLAB], F32, name=f"{nm}_mrb", tag="mmout")
                nc.tensor.matmul(mrb_ps[:], onesrow_t[:], mrbf[:])
                rb = a1.tile([P, SLAB], BF16, name=f"{nm}_rbs", tag="ln_rbs", bufs=1)
                nc.scalar.activation(rb[:], rb_ps[:], AF.Copy)
                mrb = a1.tile([P, SLAB], BF16, name=f"{nm}_mrbs", tag="ln_mrbs", bufs=1)
                nc.scalar.activation(mrb[:], mrb_ps[:], AF.Copy)
                y = ypool.tile([P, KC * SLAB], BF16, name=f"{nm}_y", tag=ytag)
                tmp = a1.tile([P, SLAB], BF16, name=f"{nm}_tmp", tag="ln_tmp", bufs=1)
                for k in range(KC):
                    sl = slice(k * SLAB, (k + 1) * SLAB)
                    nc.vector.tensor_mul(tmp[:], h_t[:, sl], rb[:])
                    nc.vector.tensor_sub(y[:, sl], tmp[:], mrb[:])
                return y

            QK2 = 2 * KC * SLAB

            def emit_qkv(c, w):
                y = c["y"]
                qk = a2.tile([P, 2 * QK2], BF16, name="qk", tag="qk", bufs=1)
                for m in range(2 * KC):
                    ps = ps2.tile([P, SLAB], F32, name=f"qkps{m}", tag="mmout")
                    for k in range(KC):
                        nc.tensor.matmul(ps[:], w["wqkv"][k][:, m * P:(m + 1) * P],
                                         y[:, k * SLAB:(k + 1) * SLAB],
                                         start=(k == 0), stop=(k == KC - 1))
                    nc.scalar.activation(qk[:, m * SLAB:(m + 1) * SLAB], ps[:],
                                         AF.Identity, bias=w["bqk"][:, m:m + 1])
                    nc.sync.dma_start(out=qk[0:64, QK2 + m * SLAB:QK2 + (m + 1) * SLAB],
                                      in_=qk[64:128, m * SLAB:(m + 1) * SLAB])
                v = a2.tile([P, KC * SLAB], BF16, name="v", tag="v", bufs=1)
                for tch in range(TC):
                    ps = ps2.tile([P, CHID], F32, name=f"vps{tch}", tag="mmout")
                    for k in range(KC):
                        nc.tensor.matmul(ps[:], y[:, k * SLAB + tch * P: k * SLAB + (tch + 1) * P],
                                         w["wqkv"][k][:, 2 * CHID:3 * CHID],
                                         start=(k == 0), stop=False)
                    nc.tensor.matmul(ps[:], onesrow_t[:], w["bvrow"][:],
                                     start=False, stop=True)
                    nc.scalar.activation(v[:, tch * SLAB:(tch + 1) * SLAB], ps[:], AF.Copy)
                c["qk"], c["v"] = qk, v
                c["ot"] = a2.tile([P, KC * SLAB], BF16, name="ot", tag="ot")

            def emit_attn_tch(c, tch, fill_fn):
                """One 128-token attention chunk; fill_fn() emits independent
                PE work between the softmax (vector/scalar) chain and the
                transposes that depend on it."""
                qk, v, ot = c["qk"], c["v"], c["ot"]
                s_ps = ps1.tile([P, NH * P], F32, name=f"s{tch}", tag="S")
                for h in range(NH):
                    hoff = 0 if h % 2 == 0 else QK2
                    qsl = qk[0:64, hoff + (h // 2) * SLAB + tch * P:
                             hoff + (h // 2) * SLAB + (tch + 1) * P]
                    ksl = qk[0:64, hoff + (KC + h // 2) * SLAB + tch * P:
                             hoff + (KC + h // 2) * SLAB + (tch + 1) * P]
                    nc.tensor.matmul(s_ps[:, h * P:(h + 1) * P], qsl, ksl)
                em = a1.tile([P, NH * P], BF16, name=f"e{tch}", tag="E")
                nc.scalar.activation(em[:], s_ps[:], AF.Exp, scale=float(HD) ** -0.5)
                sums = a1.tile([P, NH], F32, name=f"sums{tch}", tag="sums")
                for h in range(NH):
                    nc.vector.scalar_tensor_tensor(em[:, h * P:(h + 1) * P],
                                                   em[:, h * P:(h + 1) * P], 1.0,
                                                   mask_t[:],
                                                   op0=ALU.mult, op1=ALU.mult,
                                                   accum_out=sums[:, h:h + 1])
                rec = a1.tile([P, NH], F32, name=f"rec{tch}", tag="rec")
                nc.vector.reciprocal_approx_fast(out=rec[:], in_=sums[:])
                for h in range(NH):
                    nc.vector.tensor_scalar(em[:, h * P:(h + 1) * P],
                                            em[:, h * P:(h + 1) * P],
                                            rec[:, h:h + 1], None, op0=ALU.mult)
                fill_fn()  # keep the PE fed while the softmax chain drains
                pt_ps = ps1.tile([P, NH * P], BF16, name=f"ptps{tch}", tag="PT")
                for h in range(NH):
                    nc.tensor.transpose(pt_ps[:, h * P:(h + 1) * P],
                                        em[:, h * P:(h + 1) * P], ident_t[:])
                pt = a1.tile([P, NH * P], BF16, name=f"pt{tch}", tag="PTS")
                nc.scalar.activation(pt[:], pt_ps[:], AF.Copy)
                o_ps = ps1.tile([P, SLAB], F32, name=f"ops{tch}", tag="OT")
                for h in range(NH):
                    nc.tensor.matmul(o_ps[64 * (h % 2):64 * (h % 2) + 64,
                                          (h // 2) * P:(h // 2 + 1) * P],
                                     v[:, tch * SLAB + 64 * h:tch * SLAB + 64 * h + 64],
                                     pt[:, h * P:(h + 1) * P])
                nc.scalar.activation(ot[:, tch * SLAB:(tch + 1) * SLAB], o_ps[:], AF.Copy)

            def emit_proj(c, w):
                x1 = a1.tile([P, KC * SLAB], BF16, name="x1", tag="x1", bufs=3)
                ot_r = c["ot"][:].rearrange("p (t j q) -> p t j q", t=TC, j=KC, q=P)
                for m in range(KC):
                    ps = ps2.tile([P, SLAB], F32, name=f"pps{m}", tag="mmout")
                    for k in range(KC):
                        nc.tensor.matmul(ps[:], w["wpw"][k][:, m * P:(m + 1) * P],
                                         ot_r[:, :, k, :],
                                         start=(k == 0), stop=(k == KC - 1))
                    nc.vector.scalar_tensor_tensor(x1[:, m * SLAB:(m + 1) * SLAB], ps[:],
                                                   w["bpb"][:, m:m + 1],
                                                   c["h"][:, m * SLAB:(m + 1) * SLAB],
                                                   op0=ALU.add, op1=ALU.add)
                c["x1"] = x1

            def emit_m1(c, w):
                c["g"] = a1.tile([P, (CH4 // P) * SLAB], BF16, name="g", tag="g", bufs=2)
                g, z = c["g"], c["z"]
                for m in range(CH4 // P):
                    ps = ps2.tile([P, SLAB], F32, name=f"m1ps{m}", tag="mmout")
                    for k in range(KC):
                        nc.tensor.matmul(ps[:], w["wm1"][k][:, m * P:(m + 1) * P],
                                         z[:, k * SLAB:(k + 1) * SLAB],
                                         start=(k == 0), stop=(k == KC - 1))
                    nc.scalar.activation(g[:, m * SLAB:(m + 1) * SLAB], ps[:], GELU_FN,
                                         bias=w["bm1"][:, m:m + 1])

            def emit_m2_group(c, m, w):
                if m == 0:
                    c["x2"] = a2.tile([P, KC * SLAB], BF16, name="x2", tag="x2")
                g, x1, x2 = c["g"], c["x1"], c["x2"]
                if True:
                    ps = ps2.tile([P, SLAB], F32, name=f"m2ps{m}", tag="mmout")
                    for k in range(CH4 // P):
                        nc.tensor.matmul(ps[:], w["wm2"][k][:, m * P:(m + 1) * P],
                                         g[:, k * SLAB:(k + 1) * SLAB],
                                         start=(k == 0), stop=(k == CH4 // P - 1))
                    nc.vector.scalar_tensor_tensor(x2[:, m * SLAB:(m + 1) * SLAB], ps[:],
                                                   w["bm2"][:, m:m + 1],
                                                   x1[:, m * SLAB:(m + 1) * SLAB],
                                                   op0=ALU.add, op1=ALU.add)

            # ===============================================================
            # 3-deep software pipeline per phase:
            #   front(i): load/ff1 + LN1 stats      (PE-dense + stats)
            #   body(i-1): qkv/V, attention, proj, LN2
            #   tail(i-2): m1 (inside attention), m2, store
            # so every serial LN/softmax chain runs while the PE chews
            # matmuls of a neighboring slab.
            # ===============================================================
            def run_phase(nsl, front_fn, store_fn, w):
                ctxs = {}
                for i in range(nsl + 3):
                    if i < nsl:
                        c = ctxs[i] = {}
                        front_fn(i, c)                      # -> c["h"]
                        c["st1"] = ln_stats(c["h"], f"l1_{i}")
                    cb = ctxs[i - 1] if 0 <= i - 1 < nsl else None
                    if cb is not None:
                        emit_qkv(cb, w)
                    if i < nsl:
                        ctxs[i]["y"] = ln_apply(ctxs[i]["h"], ctxs[i]["st1"],
                                                f"l1_{i}", a2, "y_bf")
                    cm2 = ctxs.get(i - 3)   # m2: vector evictions only, so it
                    for tch in range(TC):   # can fill attention's PE idle time
                        if cb is not None:
                            fill = (lambda c2=cm2, t2=tch: emit_m2_group(c2, t2, w)) \
                                if cm2 is not None else (lambda: None)
                            emit_attn_tch(cb, tch, fill)
                        elif cm2 is not None:
                            emit_m2_group(cm2, tch, w)
                    if cb is not None:
                        emit_proj(cb, w)
                    cm1 = ctxs.get(i - 2)   # m1 contiguous: one Gelu table load
                    if cm1 is not None:
                        emit_m1(cm1, w)
                    if cm2 is not None:
                        store_fn(i - 3, cm2)
                        del ctxs[i - 3]
                    if cb is not None:
                        cb["st2"] = ln_stats(cb["x1"], f"l2_{i-1}")
                        cb["z"] = ln_apply(cb["x1"], cb["st2"], f"l2_{i-1}", a1, "z_bf")

            # ---- phase 1: x -> ff1 -> block A -> h1 scratch ----
            t_w = load_block_weights("a")

            def front1(b, c):
                t0 = b * SLAB
                xq = a1.tile([P, JC * SLAB], BF16, name=f"xq{b}", tag="xq")
                for j in range(JC):
                    nc.sync.dma_start(out=xq[:, j * SLAB:(j + 1) * SLAB],
                                      in_=x_in[j][:, t0:t0 + SLAB])
                h_t = a2.tile([P, KC * SLAB], BF16, name=f"h{b}", tag="h")
                for m in range(KC):
                    ps = ps2.tile([P, SLAB], F32, name=f"f1ps{b}_{m}", tag="mmout")
                    for j in range(JC):
                        nc.tensor.matmul(ps[:],
                                         wff1_t[:, j * CHID + m * P:j * CHID + (m + 1) * P],
                                         xq[:, j * SLAB:(j + 1) * SLAB],
                                         start=(j == 0), stop=(j == JC - 1))
                    nc.scalar.activation(h_t[:, m * SLAB:(m + 1) * SLAB], ps[:], GELU_FN,
                                         bias=bff1_t[:, m:m + 1])
                c["h"] = h_t

            def store1(b, c):
                t0 = b * SLAB
                for k in range(KC):
                    nc.sync.dma_start(out=h1_t[k, :, t0:t0 + SLAB],
                                      in_=c["x2"][:, k * SLAB:(k + 1) * SLAB])

            run_phase(cfg.n1, front1, store1, t_w)

            # ---- phase 2: h1 -> block B -> ff2 -> out ----
            t_w = load_block_weights("b")

            def front2(b, c):
                c0 = b * SLAB + HALO - WS // 2  # slab origin in scratch coords
                h_t = a2.tile([P, KC * SLAB], BF16, name=f"hb{b}", tag="h")
                for k in range(KC):
                    nc.sync.dma_start(out=h_t[:, k * SLAB:(k + 1) * SLAB],
                                      in_=h1_t[k, :, c0:c0 + SLAB])
                c["h"] = h_t

            def store2(b, c):
                x2 = c["x2"]
                o_t = a1.tile([P, TC * COUT], F32, name=f"o{b}", tag="o", bufs=1)
                out_base = b * SLAB - WS // 2  # first out row this slab covers
                for tch in range(TC):
                    r0 = out_base + tch * P  # out rows [r0, r0+128)
                    lo, hi = max(r0, 0), min(r0 + P, cfg.t_out)
                    if lo >= hi:
                        continue
                    ps = ps2.tile([P, COUT], F32, name=f"f2ps{b}_{tch}", tag="mmout")
                    for k in range(KC):
                        nc.tensor.matmul(ps[:],
                                         x2[:, k * SLAB + tch * P:k * SLAB + (tch + 1) * P],
                                         wff2_t[:, k * COUT:(k + 1) * COUT],
                                         start=(k == 0), stop=False)
                    nc.tensor.matmul(ps[:], onesrow_t[:],
                                     bff2_t[:], start=False, stop=True)
                    nc.scalar.activation(o_t[:, tch * COUT:(tch + 1) * COUT], ps[:], AF.Copy)
                    nc.sync.dma_start(out=out[lo:hi, :],
                                      in_=o_t[lo - r0:hi - r0, tch * COUT:(tch + 1) * COUT])

            run_phase(cfg.n2, front2, store2, t_w)

    nc.compile()
    return nc


# ---------------------------------------------------------------------------
# host-side input preparation
# ---------------------------------------------------------------------------


def _sigma(W, u):
    W = np.asarray(W, np.float32)
    u = np.asarray(u, np.float32)
    v = W @ u
    v = v / (np.linalg.norm(v) + 1e-12)
    u2 = v @ W
    u2 = u2 / (np.linalg.norm(u2) + 1e-12)
    return float(v @ W @ u2)


def prep_weights(inputs):
    """Host-side: fold LN affine + spectral norm into weights; tile/cast."""
    f32 = np.float32
    d = {}
    w1 = np.asarray(inputs["ff1_w"], f32)
    d["wff1"] = _to_bf16(w1.reshape(JC, P, CHID))
    d["bff1"] = np.ascontiguousarray(np.asarray(inputs["ff1_b"], f32).reshape(KC, P).T)

    sig = _sigma(inputs["ff2_w"], inputs["ff2_u"])
    w2 = np.asarray(inputs["ff2_w"], f32) / sig
    d["wff2"] = _to_bf16(w2.reshape(KC, P, COUT))
    d["bff2row"] = _to_bf16(np.asarray(inputs["ff2_b"], f32).reshape(1, COUT))

    for p in ("a", "b"):
        g1 = np.asarray(inputs[f"{p}_ln1g"], f32)
        b1 = np.asarray(inputs[f"{p}_ln1b"], f32)
        qkvw = np.asarray(inputs[f"{p}_qkvw"], f32)
        qkvb = np.asarray(inputs[f"{p}_qkvb"], f32) + b1 @ qkvw
        wg = g1[:, None] * qkvw  # [512, 1536]
        d[f"{p}_wqkv"] = _to_bf16(wg.reshape(KC, P, 3 * CHID))
        d[f"{p}_bqk"] = np.ascontiguousarray(qkvb[:2 * CHID].reshape(2 * KC, P).T)
        d[f"{p}_bvrow"] = _to_bf16(qkvb[2 * CHID:].reshape(1, CHID))
        pw = np.asarray(inputs[f"{p}_pw"], f32)
        d[f"{p}_wpw"] = _to_bf16(pw.reshape(KC, P, CHID))
        d[f"{p}_bpb"] = np.ascontiguousarray(np.asarray(inputs[f"{p}_pb"], f32).reshape(KC, P).T)
        g2 = np.asarray(inputs[f"{p}_ln2g"], f32)
        b2 = np.asarray(inputs[f"{p}_ln2b"], f32)
        m1w = np.asarray(inputs[f"{p}_m1w"], f32)
        m1b = np.asarray(inputs[f"{p}_m1b"], f32) + b2 @ m1w
        d[f"{p}_wm1"] = _to_bf16((g2[:, None] * m1w).reshape(KC, P, CH4))
        d[f"{p}_bm1"] = np.ascontiguousarray(m1b.reshape(CH4 // P, P).T)
        m2w = np.asarray(inputs[f"{p}_m2w"], f32)
        d[f"{p}_wm2"] = _to_bf16(m2w.reshape(CH4 // P, P, CHID))
        d[f"{p}_bm2"] = np.ascontiguousarray(np.asarray(inputs[f"{p}_m2b"], f32).reshape(KC, P).T)

    d["ident_bf"] = _to_bf16(np.eye(P, dtype=f32))
    m = np.zeros((P, P), f32)
    for wdw in range(P // WS):
        m[wdw * WS:(wdw + 1) * WS, wdw * WS:(wdw + 1) * WS] = 1.0
    d["mask_bf"] = _to_bf16(m)
    d["ones_col_bf"] = _to_bf16(np.ones((P, 1), f32))
    d["ones_row_bf"] = _to_bf16(np.ones((1, P), f32))
    d["eps_t"] = np.full((1, 1), 1e-5, f32)
    return d


def _to_bf16(a):
    import ml_dtypes
    return np.ascontiguousarray(np.asarray(a, np.float32)).astype(ml_dtypes.bfloat16)


def make_in_maps(x, wd, cfg: Cfg):
    """x: [B, N, CIN]. Returns per-core input maps (x pre-transposed, bf16)."""
    maps = []
    cores_per_batch = max(1, cfg.n_cores // x.shape[0])
    for c in range(cfg.n_cores):
        beta = c // cores_per_batch
        eta = c % cores_per_batch
        start = eta * cfg.t_out - HALO
        idx = (start + np.arange(cfg.t_in)) % x.shape[1]
        xt = np.ascontiguousarray(x[beta, idx].T).reshape(JC, P, cfg.t_in)
        m = {"x_in": _to_bf16(xt)}
        m.update(wd)
        maps.append(m)
    return maps


_PROG = {}


def _get_prog(cfg: Cfg):
    key = (cfg.t_out, cfg.n_cores)
    if key not in _PROG:
        _PROG[key] = build_program(cfg)
    return _PROG[key]


def kernel(**inputs) -> np.ndarray:
    x = np.asarray(inputs["x"], np.float32)
    Bx, Nx = x.shape[0], x.shape[1]
    n_cores = 8
    cores_per_batch = n_cores // Bx
    cfg = Cfg(t_out=Nx // cores_per_batch, n_cores=n_cores)
    nc = _get_prog(cfg)
    wd = prep_weights(inputs)
    in_maps = make_in_maps(x, wd, cfg)
    res = run_bass_kernel_spmd(nc, in_maps, core_ids=list(range(n_cores)))
    out = np.empty((Bx, Nx, COUT), np.float32)
    for c in range(n_cores):
        beta = c // cores_per_batch
        eta = c % cores_per_batch
        out[beta, eta * cfg.t_out:(eta + 1) * cfg.t_out] = res.results[c]["out"]
    return out



# revision 3
# speedup vs baseline: 1.5094x; 1.5094x over previous
"""Trainium2 Bass kernel for nn_Block_77481210020339 (HEALPix Swin-style block).

Pipeline: ff1(256->512)+gelu -> [LN -> win16-attn -> +res -> LN -> MLP(512->2048->512) -> +res] (block A)
          -> same shifted by ws/2=8 (block B) -> spectral-norm linear (512->256).

Sharding: 8 cores, each takes half of one batch image (24576 of 49152 pixels)
plus a redundantly-computed halo, so no collectives are needed.  Each core runs
two phases over 512-token slabs:
  phase 1: x -> ff1 -> block A -> h1 (HBM scratch, bf16 [c,t] layout)
  phase 2: h1 -> block B (shifted windows = aligned windows on slabs offset -8)
           -> ff2 (spectral-norm folded on host) -> out

Throughput rework vs the f32 baseline (device exec 12.6ms -> ~8ms/core):
  - x is pre-transposed to [c,t] and cast to bf16 on the host: no PE input
    transposes, all matmuls bf16 (FWL weight loads).
  - the whole residual stream and HBM scratch are bf16 (f32 only inside PSUM
    and LN statistics), doubling DVE throughput and halving DMA bytes.
  - attention: one batched Exp over all 8 heads' scores [128,1024], bf16
    probabilities, single batched P^T eviction, fast reciprocal.
  - LN split into stats (PE matmuls) and a serial scalar/vector tail that is
    emitted only after independent PE work (the PE queue is in-order, so a
    matmul waiting on the tail would head-of-line-block everything after it).
  - 4-deep software pipeline per phase: front/LN1(i), qkv+attention(i-1)
    with m2(i-3) matmuls interleaved into the softmax chain (m2 evicts on
    the vector engine, so the scalar queue - busy with Exp + act-table
    loads - never gates PSUM recycling), then proj(i-1), m1(i-2) as one
    contiguous Gelu block, LN2(i-1).  Keeps the PE fed through every
    LN/softmax chain so the HAM clock gate stays at 8/8.
  - exact 8-bank PSUM plan: mmout x2, stats x2, S x2, PT(bf16) x1, OT x1.
"""

import sys

sys.path.insert(0, "/opt/trn_rl_repo")

from contextlib import ExitStack

import numpy as np

import concourse.bass as bass
from concourse import bacc
import concourse.mybir as mybir
import concourse.tile as tile

F32 = mybir.dt.float32
BF16 = mybir.dt.bfloat16
AF = mybir.ActivationFunctionType
ALU = mybir.AluOpType
GELU_FN = AF.Gelu

# problem dims
B, N, CIN, CHID, COUT, WS, NH, HD = 4, 49152, 256, 512, 256, 16, 8, 64
CH4 = 4 * CHID  # 2048
P = 128
SLAB = 512
KC = CHID // P  # 4 channel chunks of the stream
JC = CIN // P   # 2 input channel chunks
TC = SLAB // P  # 4 token chunks per slab
HALO = 256


class Cfg:
    def __init__(self, t_out=24576, n_cores=8):
        self.t_out = t_out
        self.t_in = t_out + 2 * SLAB  # phase-1 scratch tokens
        self.n1 = self.t_in // SLAB  # phase-1 slabs
        self.n2 = t_out // SLAB + 1  # phase-2 slabs
        self.n_cores = n_cores


# ---------------------------------------------------------------------------
# program builder
# ---------------------------------------------------------------------------


def build_program(cfg: Cfg):
    nc = bacc.Bacc("TRN2", target_bir_lowering=False, debug=False,
                   enable_asserts=True, num_devices=cfg.n_cores)

    # ---- external params -------------------------------------------------
    x_in = nc.declare_dram_parameter("x_in", [JC, P, cfg.t_in], BF16, isOutput=False)
    wff1 = nc.declare_dram_parameter("wff1", [JC, P, CHID], BF16, isOutput=False)
    bff1 = nc.declare_dram_parameter("bff1", [P, KC], F32, isOutput=False)
    wff2 = nc.declare_dram_parameter("wff2", [KC, P, COUT], BF16, isOutput=False)
    bff2row = nc.declare_dram_parameter("bff2row", [1, COUT], BF16, isOutput=False)

    blk = {}
    for p in ("a", "b"):
        blk[p] = dict(
            wqkv=nc.declare_dram_parameter(f"{p}_wqkv", [KC, P, 3 * CHID], BF16, isOutput=False),
            bqk=nc.declare_dram_parameter(f"{p}_bqk", [P, 2 * KC], F32, isOutput=False),
            bvrow=nc.declare_dram_parameter(f"{p}_bvrow", [1, CHID], BF16, isOutput=False),
            wpw=nc.declare_dram_parameter(f"{p}_wpw", [KC, P, CHID], BF16, isOutput=False),
            bpb=nc.declare_dram_parameter(f"{p}_bpb", [P, KC], F32, isOutput=False),
            wm1=nc.declare_dram_parameter(f"{p}_wm1", [KC, P, CH4], BF16, isOutput=False),
            bm1=nc.declare_dram_parameter(f"{p}_bm1", [P, CH4 // P], F32, isOutput=False),
            wm2=nc.declare_dram_parameter(f"{p}_wm2", [CH4 // P, P, CHID], BF16, isOutput=False),
            bm2=nc.declare_dram_parameter(f"{p}_bm2", [P, KC], F32, isOutput=False),
        )

    ident_bf = nc.declare_dram_parameter("ident_bf", [P, P], BF16, isOutput=False)
    mask_bf = nc.declare_dram_parameter("mask_bf", [P, P], BF16, isOutput=False)
    ones_col_bf = nc.declare_dram_parameter("ones_col_bf", [P, 1], BF16, isOutput=False)
    ones_row_bf = nc.declare_dram_parameter("ones_row_bf", [1, P], BF16, isOutput=False)
    eps_t = nc.declare_dram_parameter("eps_t", [1, 1], F32, isOutput=False)

    out = nc.declare_dram_parameter("out", [cfg.t_out, COUT], F32, isOutput=True)

    ctx = ExitStack()
    with tile.TileContext(nc) as tc:
        with ctx:
            cpool = ctx.enter_context(tc.tile_pool(name="const", bufs=1))
            wpool = ctx.enter_context(tc.tile_pool(name="w", bufs=1))
            a1 = ctx.enter_context(tc.tile_pool(name="a1", bufs=2))
            a2 = ctx.enter_context(tc.tile_pool(name="a2", bufs=2))
            dpool = ctx.enter_context(tc.tile_pool(name="dram", bufs=1, space="DRAM"))
            ps2 = ctx.enter_context(tc.tile_pool(name="ps2", bufs=2, space="PSUM"))
            ps1 = ctx.enter_context(tc.tile_pool(name="ps1", bufs=1, space="PSUM"))

            # ---- constants ----
            ident_t = cpool.tile([P, P], BF16, name="ident_t")
            nc.sync.dma_start(out=ident_t[:], in_=ident_bf[:])
            mask_t = cpool.tile([P, P], BF16, name="mask_t")
            nc.sync.dma_start(out=mask_t[:], in_=mask_bf[:])
            onescol_t = cpool.tile([P, 1], BF16, name="onescol_t")
            nc.sync.dma_start(out=onescol_t[:], in_=ones_col_bf[:])
            onesrow_t = cpool.tile([1, P], BF16, name="onesrow_t")
            nc.sync.dma_start(out=onesrow_t[:], in_=ones_row_bf[:])
            epsc_t = cpool.tile([1, 1], F32, name="epsc_t")
            nc.sync.dma_start(out=epsc_t[:], in_=eps_t[:])
            wff1_t = cpool.tile([P, JC * CHID], BF16, name="wff1_t")
            for j in range(JC):
                nc.sync.dma_start(out=wff1_t[:, j * CHID:(j + 1) * CHID], in_=wff1[j])
            bff1_t = cpool.tile([P, KC], F32, name="bff1_t")
            nc.sync.dma_start(out=bff1_t[:], in_=bff1[:])
            wff2_t = cpool.tile([P, KC * COUT], BF16, name="wff2_t")
            for j in range(KC):
                nc.sync.dma_start(out=wff2_t[:, j * COUT:(j + 1) * COUT], in_=wff2[j])
            bff2_t = cpool.tile([1, COUT], BF16, name="bff2_t")
            nc.sync.dma_start(out=bff2_t[:], in_=bff2row[:])

            # ---- HBM scratch for block-A output (bf16) ----
            h1_t = dpool.tile([KC, P, cfg.t_in], BF16, name="h1_t")

            # ---------------------------------------------------------------
            def load_block_weights(p):
                w = blk[p]
                t = {}
                t["wqkv"] = [wpool.tile([P, 3 * CHID], BF16, name=f"wqkv{k}", tag=f"wqkv{k}") for k in range(KC)]
                for k in range(KC):
                    nc.sync.dma_start(out=t["wqkv"][k][:], in_=w["wqkv"][k])
                t["bqk"] = wpool.tile([P, 2 * KC], F32, name="bqk", tag="bqk")
                nc.sync.dma_start(out=t["bqk"][:], in_=w["bqk"][:])
                t["bvrow"] = wpool.tile([1, CHID], BF16, name="bvrow", tag="bvrow")
                nc.sync.dma_start(out=t["bvrow"][:], in_=w["bvrow"][:])
                t["wpw"] = [wpool.tile([P, CHID], BF16, name=f"wpw{k}", tag=f"wpw{k}") for k in range(KC)]
                for k in range(KC):
                    nc.sync.dma_start(out=t["wpw"][k][:], in_=w["wpw"][k])
                t["bpb"] = wpool.tile([P, KC], F32, name="bpb", tag="bpb")
                nc.sync.dma_start(out=t["bpb"][:], in_=w["bpb"][:])
                t["wm1"] = [wpool.tile([P, CH4], BF16, name=f"wm1{k}", tag=f"wm1{k}") for k in range(KC)]
                for k in range(KC):
                    nc.sync.dma_start(out=t["wm1"][k][:], in_=w["wm1"][k])
                t["bm1"] = wpool.tile([P, CH4 // P], F32, name="bm1", tag="bm1")
                nc.sync.dma_start(out=t["bm1"][:], in_=w["bm1"][:])
                t["wm2"] = [wpool.tile([P, CHID], BF16, name=f"wm2{k}", tag=f"wm2{k}") for k in range(CH4 // P)]
                for k in range(CH4 // P):
                    nc.sync.dma_start(out=t["wm2"][k][:], in_=w["wm2"][k])
                t["bm2"] = wpool.tile([P, KC], F32, name="bm2", tag="bm2")
                nc.sync.dma_start(out=t["bm2"][:], in_=w["bm2"][:])
                return t

            # ---------------------------------------------------------------
            # LayerNorm split in two so the serial scalar/vector tail can be
            # emitted AFTER independent PE work (the PE queue is in-order; a
            # matmul that waits on the tail would head-of-line-block every
            # matmul emitted after it).
            def ln_stats(h_t, nm):
                sq = a1.tile([P, KC * SLAB], BF16, name=f"{nm}_sq", tag="ln_sq", bufs=1)
                for k in range(KC):
                    sl = slice(k * SLAB, (k + 1) * SLAB)
                    nc.vector.tensor_mul(sq[:, sl], h_t[:, sl], h_t[:, sl])
                st = ps1.tile([64, SLAB], F32, name=f"{nm}_st", tag="stats", bufs=2)
                for k in range(KC):
                    sl = slice(k * SLAB, (k + 1) * SLAB)
                    nc.tensor.matmul(st[0:1, :], onescol_t[:], h_t[:, sl],
                                     start=(k == 0), stop=(k == KC - 1))
                for k in range(KC):
                    sl = slice(k * SLAB, (k + 1) * SLAB)
                    nc.tensor.matmul(st[32:33, :], onescol_t[:], sq[:, sl],
                                     start=(k == 0), stop=(k == KC - 1))
                return st

            def ln_apply(h_t, st, nm, ypool, ytag):
                m_row = a1.tile([1, SLAB], F32, name=f"{nm}_m", tag="ln_m", bufs=1)
                nc.vector.tensor_scalar(m_row[:], st[0:1, :], 1.0 / CHID, None,
                                        op0=ALU.mult)
                ms = a1.tile([1, SLAB], F32, name=f"{nm}_ms", tag="ln_ms", bufs=1)
                nc.vector.tensor_mul(ms[:], m_row[:], m_row[:])
                var = a1.tile([1, SLAB], F32, name=f"{nm}_var", tag="ln_var", bufs=1)
                nc.vector.scalar_tensor_tensor(var[:], st[32:33, :], 1.0 / CHID, ms[:],
                                               op0=ALU.mult, op1=ALU.subtract)
                std = a1.tile([1, SLAB], F32, name=f"{nm}_std", tag="ln_std", bufs=1)
                nc.scalar.activation(std[:], var[:], AF.Sqrt, bias=epsc_t[0:1, 0:1])
                r = a1.tile([1, SLAB], F32, name=f"{nm}_r", tag="ln_r", bufs=1)
                nc.vector.reciprocal_approx_fast(out=r[:], in_=std[:])
                rbf = a1.tile([1, SLAB], BF16, name=f"{nm}_rbf", tag="ln_rbf", bufs=1)
                nc.vector.tensor_scalar(rbf[:], r[:], 1.0, None, op0=ALU.mult)
                mrbf = a1.tile([1, SLAB], BF16, name=f"{nm}_mrbf", tag="ln_mrbf", bufs=1)
                nc.vector.scalar_tensor_tensor(mrbf[:], st[0:1, :], 1.0 / CHID, r[:],
                                               op0=ALU.mult, op1=ALU.mult)
                rb_ps = ps2.tile([P, SLAB], F32, name=f"{nm}_rb", tag="mmout")
                nc.tensor.matmul(rb_ps[:], onesrow_t[:], rbf[:])
                mrb_ps = ps2.tile([P, SLAB], F32, name=f"{nm}_mrb", tag="mmout")
                nc.tensor.matmul(mrb_ps[:], onesrow_t[:], mrbf[:])
                rb = a1.tile([P, SLAB], BF16, name=f"{nm}_rbs", tag="ln_rbs", bufs=1)
                nc.scalar.activation(rb[:], rb_ps[:], AF.Copy)
                mrb = a1.tile([P, SLAB], BF16, name=f"{nm}_mrbs", tag="ln_mrbs", bufs=1)
                nc.scalar.activation(mrb[:], mrb_ps[:], AF.Copy)
                y = ypool.tile([P, KC * SLAB], BF16, name=f"{nm}_y", tag=ytag)
                tmp = a1.tile([P, SLAB], BF16, name=f"{nm}_tmp", tag="ln_tmp", bufs=1)
                for k in range(KC):
                    sl = slice(k * SLAB, (k + 1) * SLAB)
                    nc.vector.tensor_mul(tmp[:], h_t[:, sl], rb[:])
                    nc.vector.tensor_sub(y[:, sl], tmp[:], mrb[:])
                return y

            QK2 = 2 * KC * SLAB

            def emit_qkv(c, w):
                y = c["y"]
                qk = a2.tile([P, 2 * QK2], BF16, name="qk", tag="qk", bufs=1)
                for m in range(2 * KC):
                    ps = ps2.tile([P, SLAB], F32, name=f"qkps{m}", tag="mmout")
                    for k in range(KC):
                        nc.tensor.matmul(ps[:], w["wqkv"][k][:, m * P:(m + 1) * P],
                                         y[:, k * SLAB:(k + 1) * SLAB],
                                         start=(k == 0), stop=(k == KC - 1))
                    nc.scalar.activation(qk[:, m * SLAB:(m + 1) * SLAB], ps[:],
                                         AF.Identity, bias=w["bqk"][:, m:m + 1])
                    nc.sync.dma_start(out=qk[0:64, QK2 + m * SLAB:QK2 + (m + 1) * SLAB],
                                      in_=qk[64:128, m * SLAB:(m + 1) * SLAB])
                v = a2.tile([P, KC * SLAB], BF16, name="v", tag="v", bufs=1)
                for tch in range(TC):
                    ps = ps2.tile([P, CHID], F32, name=f"vps{tch}", tag="mmout")
                    for k in range(KC):
                        nc.tensor.matmul(ps[:], y[:, k * SLAB + tch * P: k * SLAB + (tch + 1) * P],
                                         w["wqkv"][k][:, 2 * CHID:3 * CHID],
                                         start=(k == 0), stop=False)
                    nc.tensor.matmul(ps[:], onesrow_t[:], w["bvrow"][:],
                                     start=False, stop=True)
                    nc.scalar.activation(v[:, tch * SLAB:(tch + 1) * SLAB], ps[:], AF.Copy)
                c["qk"], c["v"] = qk, v
                c["ot"] = a2.tile([P, KC * SLAB], BF16, name="ot", tag="ot")

            def emit_attn_tch(c, tch, fill_fn):
                """One 128-token attention chunk; fill_fn() emits independent
                PE work between the softmax (vector/scalar) chain and the
                transposes that depend on it."""
                qk, v, ot = c["qk"], c["v"], c["ot"]
                s_ps = ps1.tile([P, NH * P], F32, name=f"s{tch}", tag="S")
                for h in range(NH):
                    hoff = 0 if h % 2 == 0 else QK2
                    qsl = qk[0:64, hoff + (h // 2) * SLAB + tch * P:
                             hoff + (h // 2) * SLAB + (tch + 1) * P]
                    ksl = qk[0:64, hoff + (KC + h // 2) * SLAB + tch * P:
                             hoff + (KC + h // 2) * SLAB + (tch + 1) * P]
                    nc.tensor.matmul(s_ps[:, h * P:(h + 1) * P], qsl, ksl)
                em = a1.tile([P, NH * P], BF16, name=f"e{tch}", tag="E")
                nc.scalar.activation(em[:], s_ps[:], AF.Exp, scale=float(HD) ** -0.5)
                sums = a1.tile([P, NH], F32, name=f"sums{tch}", tag="sums")
                for h in range(NH):
                    nc.vector.scalar_tensor_tensor(em[:, h * P:(h + 1) * P],
                                                   em[:, h * P:(h + 1) * P], 1.0,
                                                   mask_t[:],
                                                   op0=ALU.mult, op1=ALU.mult,
                                                   accum_out=sums[:, h:h + 1])
                rec = a1.tile([P, NH], F32, name=f"rec{tch}", tag="rec")
                nc.vector.reciprocal_approx_fast(out=rec[:], in_=sums[:])
                for h in range(NH):
                    nc.vector.tensor_scalar(em[:, h * P:(h + 1) * P],
                                            em[:, h * P:(h + 1) * P],
                                            rec[:, h:h + 1], None, op0=ALU.mult)
                fill_fn()  # keep the PE fed while the softmax chain drains
                pt_ps = ps1.tile([P, NH * P], BF16, name=f"ptps{tch}", tag="PT")
                for h in range(NH):
                    nc.tensor.transpose(pt_ps[:, h * P:(h + 1) * P],
                                        em[:, h * P:(h + 1) * P], ident_t[:])
                pt = a1.tile([P, NH * P], BF16, name=f"pt{tch}", tag="PTS")
                nc.scalar.activation(pt[:], pt_ps[:], AF.Copy)
                o_ps = ps1.tile([P, SLAB], F32, name=f"ops{tch}", tag="OT")
                for h in range(NH):
                    nc.tensor.matmul(o_ps[64 * (h % 2):64 * (h % 2) + 64,
                                          (h // 2) * P:(h // 2 + 1) * P],
                                     v[:, tch * SLAB + 64 * h:tch * SLAB + 64 * h + 64],
                                     pt[:, h * P:(h + 1) * P])
                nc.scalar.activation(ot[:, tch * SLAB:(tch + 1) * SLAB], o_ps[:], AF.Copy)

            def emit_proj(c, w):
                x1 = a1.tile([P, KC * SLAB], BF16, name="x1", tag="x1", bufs=3)
                ot_r = c["ot"][:].rearrange("p (t j q) -> p t j q", t=TC, j=KC, q=P)
                for m in range(KC):
                    ps = ps2.tile([P, SLAB], F32, name=f"pps{m}", tag="mmout")
                    for k in range(KC):
                        nc.tensor.matmul(ps[:], w["wpw"][k][:, m * P:(m + 1) * P],
                                         ot_r[:, :, k, :],
                                         start=(k == 0), stop=(k == KC - 1))
                    nc.vector.scalar_tensor_tensor(x1[:, m * SLAB:(m + 1) * SLAB], ps[:],
                                                   w["bpb"][:, m:m + 1],
                                                   c["h"][:, m * SLAB:(m + 1) * SLAB],
                                                   op0=ALU.add, op1=ALU.add)
                c["x1"] = x1

            def emit_m1(c, w):
                c["g"] = a1.tile([P, (CH4 // P) * SLAB], BF16, name="g", tag="g", bufs=2)
                g, z = c["g"], c["z"]
                for m in range(CH4 // P):
                    ps = ps2.tile([P, SLAB], F32, name=f"m1ps{m}", tag="mmout")
                    for k in range(KC):
                        nc.tensor.matmul(ps[:], w["wm1"][k][:, m * P:(m + 1) * P],
                                         z[:, k * SLAB:(k + 1) * SLAB],
                                         start=(k == 0), stop=(k == KC - 1))
                    nc.scalar.activation(g[:, m * SLAB:(m + 1) * SLAB], ps[:], GELU_FN,
                                         bias=w["bm1"][:, m:m + 1])

            def emit_m2_group(c, m, w):
                if m == 0:
                    c["x2"] = a2.tile([P, KC * SLAB], BF16, name="x2", tag="x2")
                g, x1, x2 = c["g"], c["x1"], c["x2"]
                if True:
                    ps = ps2.tile([P, SLAB], F32, name=f"m2ps{m}", tag="mmout")
                    for k in range(CH4 // P):
                        nc.tensor.matmul(ps[:], w["wm2"][k][:, m * P:(m + 1) * P],
                                         g[:, k * SLAB:(k + 1) * SLAB],
                                         start=(k == 0), stop=(k == CH4 // P - 1))
                    nc.vector.scalar_tensor_tensor(x2[:, m * SLAB:(m + 1) * SLAB], ps[:],
                                                   w["bm2"][:, m:m + 1],
                                                   x1[:, m * SLAB:(m + 1) * SLAB],
                                                   op0=ALU.add, op1=ALU.add)

            # ===============================================================
            # 3-deep software pipeline per phase:
            #   front(i): load/ff1 + LN1 stats      (PE-dense + stats)
            #   body(i-1): qkv/V, attention, proj, LN2
            #   tail(i-2): m1 (inside attention), m2, store
            # so every serial LN/softmax chain runs while the PE chews
            # matmuls of a neighboring slab.
            # ===============================================================
            def run_phase(nsl, front_fn, store_fn, w):
                ctxs = {}
                for i in range(nsl + 3):
                    if i < nsl:
                        c = ctxs[i] = {}
                        front_fn(i, c)                      # -> c["h"]
                        c["st1"] = ln_stats(c["h"], f"l1_{i}")
                    cb = ctxs[i - 1] if 0 <= i - 1 < nsl else None
                    if cb is not None:
                        emit_qkv(cb, w)
                    if i < nsl:
                        ctxs[i]["y"] = ln_apply(ctxs[i]["h"], ctxs[i]["st1"],
                                                f"l1_{i}", a2, "y_bf")
                    cm2 = ctxs.get(i - 3)   # m2: vector evictions only, so it
                    for tch in range(TC):   # can fill attention's PE idle time
                        if cb is not None:
                            fill = (lambda c2=cm2, t2=tch: emit_m2_group(c2, t2, w)) \
                                if cm2 is not None else (lambda: None)
                            emit_attn_tch(cb, tch, fill)
                        elif cm2 is not None:
                            emit_m2_group(cm2, tch, w)
                    if cb is not None:
                        emit_proj(cb, w)
                    cm1 = ctxs.get(i - 2)   # m1 contiguous: one Gelu table load
                    if cm1 is not None:
                        emit_m1(cm1, w)
                    if cm2 is not None:
                        store_fn(i - 3, cm2)
                        del ctxs[i - 3]
                    if cb is not None:
                        cb["st2"] = ln_stats(cb["x1"], f"l2_{i-1}")
                        cb["z"] = ln_apply(cb["x1"], cb["st2"], f"l2_{i-1}", a1, "z_bf")

            # ---- phase 1: x -> ff1 -> block A -> h1 scratch ----
            t_w = load_block_weights("a")

            def front1(b, c):
                t0 = b * SLAB
                xq = a1.tile([P, JC * SLAB], BF16, name=f"xq{b}", tag="xq")
                for j in range(JC):
                    nc.sync.dma_start(out=xq[:, j * SLAB:(j + 1) * SLAB],
                                      in_=x_in[j][:, t0:t0 + SLAB])
                h_t = a2.tile([P, KC * SLAB], BF16, name=f"h{b}", tag="h")
                for m in range(KC):
                    ps = ps2.tile([P, SLAB], F32, name=f"f1ps{b}_{m}", tag="mmout")
                    for j in range(JC):
                        nc.tensor.matmul(ps[:],
                                         wff1_t[:, j * CHID + m * P:j * CHID + (m + 1) * P],
                                         xq[:, j * SLAB:(j + 1) * SLAB],
                                         start=(j == 0), stop=(j == JC - 1))
                    nc.scalar.activation(h_t[:, m * SLAB:(m + 1) * SLAB], ps[:], GELU_FN,
                                         bias=bff1_t[:, m:m + 1])
                c["h"] = h_t

            def store1(b, c):
                t0 = b * SLAB
                for k in range(KC):
                    nc.sync.dma_start(out=h1_t[k, :, t0:t0 + SLAB],
                                      in_=c["x2"][:, k * SLAB:(k + 1) * SLAB])

            run_phase(cfg.n1, front1, store1, t_w)

            # ---- phase 2: h1 -> block B -> ff2 -> out ----
            t_w = load_block_weights("b")

            def front2(b, c):
                c0 = b * SLAB + HALO - WS // 2  # slab origin in scratch coords
                h_t = a2.tile([P, KC * SLAB], BF16, name=f"hb{b}", tag="h")
                for k in range(KC):
                    nc.sync.dma_start(out=h_t[:, k * SLAB:(k + 1) * SLAB],
                                      in_=h1_t[k, :, c0:c0 + SLAB])
                c["h"] = h_t

            def store2(b, c):
                x2 = c["x2"]
                o_t = a1.tile([P, TC * COUT], F32, name=f"o{b}", tag="o", bufs=1)
                out_base = b * SLAB - WS // 2  # first out row this slab covers
                for tch in range(TC):
                    r0 = out_base + tch * P  # out rows [r0, r0+128)
                    lo, hi = max(r0, 0), min(r0 + P, cfg.t_out)
                    if lo >= hi:
                        continue
                    ps = ps2.tile([P, COUT], F32, name=f"f2ps{b}_{tch}", tag="mmout")
                    for k in range(KC):
                        nc.tensor.matmul(ps[:],
                                         x2[:, k * SLAB + tch * P:k * SLAB + (tch + 1) * P],
                                         wff2_t[:, k * COUT:(k + 1) * COUT],
                                         start=(k == 0), stop=False)
                    nc.tensor.matmul(ps[:], onesrow_t[:],
                                     bff2_t[:], start=False, stop=True)
                    nc.scalar.activation(o_t[:, tch * COUT:(tch + 1) * COUT], ps[:], AF.Copy)
                    nc.sync.dma_start(out=out[lo:hi, :],
                                      in_=o_t[lo - r0:hi - r0, tch * COUT:(tch + 1) * COUT])

            run_phase(cfg.n2, front2, store2, t_w)

    nc.compile()
    return nc


# ---------------------------------------------------------------------------
# host-side input preparation
# ---------------------------------------------------------------------------


def _sigma(W, u):
    W = np.asarray(W, np.float32)
    u = np.asarray(u, np.float32)
    v = W @ u
    v = v / (np.linalg.norm(v) + 1e-12)
    u2 = v @ W
    u2 = u2 / (np.linalg.norm(u2) + 1e-12)
    return float(v @ W @ u2)


def prep_weights(inputs):
    """Host-side: fold LN affine + spectral norm into weights; tile/cast."""
    f32 = np.float32
    d = {}
    w1 = np.asarray(inputs["ff1_w"], f32)
    d["wff1"] = _to_bf16(w1.reshape(JC, P, CHID))
    d["bff1"] = np.ascontiguousarray(np.asarray(inputs["ff1_b"], f32).reshape(KC, P).T)

    sig = _sigma(inputs["ff2_w"], inputs["ff2_u"])
    w2 = np.asarray(inputs["ff2_w"], f32) / sig
    d["wff2"] = _to_bf16(w2.reshape(KC, P, COUT))
    d["bff2row"] = _to_bf16(np.asarray(inputs["ff2_b"], f32).reshape(1, COUT))

    for p in ("a", "b"):
        g1 = np.asarray(inputs[f"{p}_ln1g"], f32)
        b1 = np.asarray(inputs[f"{p}_ln1b"], f32)
        qkvw = np.asarray(inputs[f"{p}_qkvw"], f32)
        qkvb = np.asarray(inputs[f"{p}_qkvb"], f32) + b1 @ qkvw
        wg = g1[:, None] * qkvw  # [512, 1536]
        d[f"{p}_wqkv"] = _to_bf16(wg.reshape(KC, P, 3 * CHID))
        d[f"{p}_bqk"] = np.ascontiguousarray(qkvb[:2 * CHID].reshape(2 * KC, P).T)
        d[f"{p}_bvrow"] = _to_bf16(qkvb[2 * CHID:].reshape(1, CHID))
        pw = np.asarray(inputs[f"{p}_pw"], f32)
        d[f"{p}_wpw"] = _to_bf16(pw.reshape(KC, P, CHID))
        d[f"{p}_bpb"] = np.ascontiguousarray(np.asarray(inputs[f"{p}_pb"], f32).reshape(KC, P).T)
        g2 = np.asarray(inputs[f"{p}_ln2g"], f32)
        b2 = np.asarray(inputs[f"{p}_ln2b"], f32)
        m1w = np.asarray(inputs[f"{p}_m1w"], f32)
        m1b = np.asarray(inputs[f"{p}_m1b"], f32) + b2 @ m1w
        d[f"{p}_wm1"] = _to_bf16((g2[:, None] * m1w).reshape(KC, P, CH4))
        d[f"{p}_bm1"] = np.ascontiguousarray(m1b.reshape(CH4 // P, P).T)
        m2w = np.asarray(inputs[f"{p}_m2w"], f32)
        d[f"{p}_wm2"] = _to_bf16(m2w.reshape(CH4 // P, P, CHID))
        d[f"{p}_bm2"] = np.ascontiguousarray(np.asarray(inputs[f"{p}_m2b"], f32).reshape(KC, P).T)

    d["ident_bf"] = _to_bf16(np.eye(P, dtype=f32))
    m = np.zeros((P, P), f32)
    for wdw in range(P // WS):
        m[wdw * WS:(wdw + 1) * WS, wdw * WS:(wdw + 1) * WS] = 1.0
    d["mask_bf"] = _to_bf16(m)
    d["ones_col_bf"] = _to_bf16(np.ones((P, 1), f32))
    d["ones_row_bf"] = _to_bf16(np.ones((1, P), f32))
    d["eps_t"] = np.full((1, 1), 1e-5, f32)
    return d


def _to_bf16(a):
    import ml_dtypes
    return np.ascontiguousarray(np.asarray(a, np.float32)).astype(ml_dtypes.bfloat16)


def make_in_maps(x, wd, cfg: Cfg):
    """x: [B, N, CIN]. Returns per-core input maps (x pre-transposed, bf16)."""
    maps = []
    cores_per_batch = max(1, cfg.n_cores // x.shape[0])
    for c in range(cfg.n_cores):
        beta = c // cores_per_batch
        eta = c % cores_per_batch
        start = eta * cfg.t_out - HALO
        idx = (start + np.arange(cfg.t_in)) % x.shape[1]
        xt = np.ascontiguousarray(x[beta, idx].T).reshape(JC, P, cfg.t_in)
        m = {"x_in": _to_bf16(xt)}
        m.update(wd)
        maps.append(m)
    return maps


_PROG = {}


def _get_prog(cfg: Cfg):
    key = (cfg.t_out, cfg.n_cores)
    if key not in _PROG:
        _PROG[key] = build_program(cfg)
    return _PROG[key]


class AsyncLauncher:
    """Per-device jit callables for a compiled bass program.

    One jit call per NeuronCore, dispatched asynchronously, so the 8 core
    programs execute CONCURRENTLY on the terminal (the single shard_map
    call run_bass_kernel_spmd makes serializes them: measured 8-core wall
    was ~overhead + 8x per-core exec; with per-device dispatch it is
    ~overhead + 1x per-core exec).
    """

    def __init__(self, nc, n_cores):
        import jax
        from concourse import bass2jax
        from concourse.bass2jax import _bass_exec_p, partition_id_tensor

        bass2jax.install_neuronx_cc_hook()
        self.jax = jax
        partition_name = (nc.partition_id_tensor.name
                          if nc.partition_id_tensor else None)
        in_names, out_names, out_avals, zero_outs = [], [], [], []
        for alloc in nc.m.functions[0].allocations:
            if not isinstance(alloc, mybir.MemoryLocationSet):
                continue
            name = alloc.memorylocations[0].name
            if alloc.kind == "ExternalInput":
                if name != partition_name:
                    in_names.append(name)
            elif alloc.kind == "ExternalOutput":
                shape = tuple(alloc.tensor_shape)
                dtype = mybir.dt.np(alloc.dtype)
                out_names.append(name)
                out_avals.append(jax.core.ShapedArray(shape, dtype))
                zero_outs.append(np.zeros(shape, dtype))
        all_in_names = list(in_names) + list(out_names)
        if partition_name is not None:
            all_in_names.append(partition_name)

        def _body(*args):
            operands = list(args)
            if partition_name is not None:
                operands.append(partition_id_tensor())
            outs = _bass_exec_p.bind(
                *operands,
                out_avals=tuple(out_avals),
                in_names=tuple(all_in_names),
                out_names=tuple(out_names),
                lowering_input_output_aliases=(),
                sim_require_finite=True,
                sim_require_nnan=True,
                nc=nc,
            )
            return tuple(outs)

        self.in_names = in_names
        self.zero_outs = zero_outs
        self.devices = jax.devices()[:n_cores]
        self.fns = [jax.jit(_body, device=dev, keep_unused=True)
                    for dev in self.devices]

    def put_args(self, in_maps):
        import jax
        dev_args = []
        for c, dev in enumerate(self.devices):
            args = [jax.device_put(np.asarray(in_maps[c][nm]), dev)
                    for nm in self.in_names]
            args += [jax.device_put(z, dev) for z in self.zero_outs]
            dev_args.append(args)
        jax.block_until_ready(dev_args)
        return dev_args

    def run(self, dev_args):
        outs = [fn(*a) for fn, a in zip(self.fns, dev_args)]
        self.jax.block_until_ready(outs)
        return outs


_LAUNCHER = {}


def _get_launcher(cfg: Cfg):
    key = (cfg.t_out, cfg.n_cores)
    if key not in _LAUNCHER:
        _LAUNCHER[key] = AsyncLauncher(_get_prog(cfg), cfg.n_cores)
    return _LAUNCHER[key]


def kernel(**inputs) -> np.ndarray:
    x = np.asarray(inputs["x"], np.float32)
    Bx, Nx = x.shape[0], x.shape[1]
    n_cores = 8
    cores_per_batch = n_cores // Bx
    cfg = Cfg(t_out=Nx // cores_per_batch, n_cores=n_cores)
    launcher = _get_launcher(cfg)
    wd = prep_weights(inputs)
    in_maps = make_in_maps(x, wd, cfg)
    dev_args = launcher.put_args(in_maps)
    outs = launcher.run(dev_args)
    out = np.empty((Bx, Nx, COUT), np.float32)
    for c in range(n_cores):
        beta = c // cores_per_batch
        eta = c % cores_per_batch
        out[beta, eta * cfg.t_out:(eta + 1) * cfg.t_out] = np.asarray(outs[c][0])
    return out

